# revision 1
# baseline (speedup 1.0000x reference)
"""MoE layer (nn_MoELayer_81630148428171) as a Trainium2 Bass kernel on 8 NeuronCores.

Strategy (data-parallel tokens + streamed expert weights, sparse top-2 compute):
  - Router runs on host (jax-cpu, bitwise-identical ops to the reference) and
    determines the *sharding*: tokens are assigned to the 8 cores with a greedy
    balancer so that every (core, expert) token count is ~equal; each core gets
    exactly 1024 tokens.
  - Each core receives its tokens pre-gathered into per-expert column segments
    (feature-major, bf16) and runs all 8 experts' FFN on just its routed tokens
    (top-2 sparse => ~2048 token-expert pairs per core):
        hidden^T = gelu(w1[e].T @ xgT_seg + b1)   (PE matmuls, bf16 in, f32 acc)
        y^T      = w2[e].T @ hidden^T + b2
    y^T tiles are PE-transposed to token-major, scaled by the combine weight and
    written to an HBM slot buffer; the final per-token output is an indirect-DMA
    gather of each token's two expert contributions plus one vector add.
  - No collectives: the host concatenates the 8 disjoint per-core token slices.
"""

import math
import numpy as np
import ml_dtypes

import concourse.bacc as bacc
import concourse.mybir as mybir
import concourse.tile as tile
from concourse.bass import IndirectOffsetOnAxis
from concourse.bass_utils import run_bass_kernel_spmd
from concourse.masks import make_identity

# Problem shapes (hardcoded per contract).
B, SEQ, H = 4, 2048, 1024
T = B * SEQ
FF = 4 * H
E = 8
TOP_K = 2
N_CORES = 8
T_PER_CORE = T // N_CORES
P = 128

BF16 = mybir.dt.bfloat16
F32 = mybir.dt.float32
I32 = mybir.dt.int32
NP_BF16 = ml_dtypes.bfloat16

_PROGRAM_CACHE: dict[int, object] = {}


# ----------------------------------------------------------------------------
# Host-side routing + sharding
# ----------------------------------------------------------------------------

def _route(x_flat, router_w, router_b):
    """Top-2 routing with bitwise-identical math to the jax reference."""
    try:
        import jax
        import jax.numpy as jnp

        cpu = jax.devices("cpu")[0]

        def f(xf, w, b):
            logits = xf @ w + b
            probs = jax.nn.softmax(logits, axis=-1)
            top_values, top_indices = jax.lax.top_k(probs, TOP_K)
            top_values = top_values / jnp.sum(top_values, axis=-1,
                                              keepdims=True)
            return top_values, top_indices

        with jax.default_device(cpu):
            tv, ti = jax.jit(f)(
                jnp.asarray(x_flat), jnp.asarray(router_w),
                jnp.asarray(router_b))
        tv = np.asarray(tv)
        ti = np.asarray(ti)
    except Exception:
        # numpy fallback (f32, same tie-breaking as lax.top_k for distinct
        # values — differences only possible for exact float ties)
        logits = x_flat @ router_w + router_b
        p = np.exp(logits - logits.max(-1, keepdims=True))
        p /= p.sum(-1, keepdims=True)
        ti = np.argsort(-p, axis=-1, kind="stable")[:, :TOP_K]
        tv = np.take_along_axis(p, ti, axis=-1)
        tv = tv / tv.sum(-1, keepdims=True)
    return (
        ti[:, 0].astype(np.int64),
        ti[:, 1].astype(np.int64),
        tv[:, 0].astype(np.float32),
        tv[:, 1].astype(np.float32),
    )


def _assign_tokens(e1, e2):
    """Greedy balanced assignment of tokens to cores.

    Keeps per-(core, expert) slot counts nearly equal while giving every core
    exactly T_PER_CORE tokens.
    """
    cnt = np.zeros((N_CORES, E), np.int64)
    tok = np.zeros(N_CORES, np.int64)
    assign = np.empty(T, np.int64)
    for t in range(T):
        a, b = e1[t], e2[t]
        best = -1
        bkey = None
        for c in range(N_CORES):
            if tok[c] >= T_PER_CORE:
                continue
            key = (cnt[c, a] + cnt[c, b], max(cnt[c, a], cnt[c, b]), tok[c])
            if bkey is None or key < bkey:
                bkey, best = key, c
        assign[t] = best
        cnt[best, a] += 1
        cnt[best, b] += 1
        tok[best] += 1
    # per-expert segment width: max over cores, padded to 4 (uniform across
    # cores, so the SPMD program can use a different width per expert)
    caps = tuple(max(4, (int(cnt[:, e].max()) + 3) // 4 * 4) for e in range(E))
    return assign, caps


def _seg_layout(caps):
    """Segment bases, total slots, per-expert sub-tile counts, cv col bases."""
    bases = [0]
    for e in range(E):
        bases.append(bases[-1] + caps[e])
    S = bases[-1]
    nsubs = [math.ceil(caps[e] / P) for e in range(E)]
    cvb = [0]
    for e in range(E):
        cvb.append(cvb[-1] + nsubs[e])
    return bases, S, nsubs, cvb


def _build_core_inputs(x_flat_bf, e1, e2, c1, c2, assign, caps, core):
    """Slot layout + device input arrays for one core."""
    bases, S, nsubs, cvb = _seg_layout(caps)
    tokens = np.nonzero(assign == core)[0]
    assert len(tokens) == T_PER_CORE

    slot_tok = np.full(S, -1, np.int64)
    cvals = np.zeros(S, np.float32)
    slotA = np.full(T_PER_CORE, 0, np.int64)
    slotB = np.full(T_PER_CORE, 0, np.int64)
    fill = np.zeros(E, np.int64)
    for i in range(T_PER_CORE):
        g = tokens[i]
        for which, (e, c) in enumerate(((e1[g], c1[g]), (e2[g], c2[g]))):
            s = bases[int(e)] + fill[e]
            fill[e] += 1
            slot_tok[s] = i
            cvals[s] = c
            if which == 0:
                slotA[i] = s
            else:
                slotB[i] = s
    assert all(fill[e] <= caps[e] for e in range(E))

    # xg pre-tiled to the SBUF layout [p, ko, slot] so the load is contiguous
    xgT = np.zeros((H, S), NP_BF16)
    valid = slot_tok >= 0
    xgT[:, valid] = x_flat_bf[tokens[slot_tok[valid]]].T
    xgp = np.ascontiguousarray(
        xgT.reshape(H // P, P, S).transpose(1, 0, 2))

    # cv: [P, sum(nsubs)]; column cvb[e]+j holds cvals[bases[e] + j*128 : +128]
    cv = np.zeros((P, cvb[-1]), np.float32)
    for e in range(E):
        for j in range(nsubs[e]):
            w = min(P, caps[e] - j * P)
            cv[:w, cvb[e] + j] = cvals[bases[e] + j * P : bases[e] + j * P + w]

    idxA = slotA.reshape(T_PER_CORE // P, P).T.astype(np.int32).copy()
    idxB = slotB.reshape(T_PER_CORE // P, P).T.astype(np.int32).copy()
    return dict(tokens=tokens, xgT=xgp, cv=cv, idxA=idxA, idxB=idxB)


# ----------------------------------------------------------------------------
# Device program
# ----------------------------------------------------------------------------

def build_program(caps, act_fn=None):
    """One SPMD program shared by all 8 cores; `caps[e]` is expert e's padded
    segment width (uniform across cores, runtime-derived compile-time const)."""
    if act_fn is None:
        act_fn = mybir.ActivationFunctionType.Gelu
    assert max(caps) <= 512, f"routing too imbalanced: {caps=}"
    bases, S, nsubs, cvb = _seg_layout(caps)
    W1_CHUNK = 512          # w1 columns per DMA chunk (4 m-tiles)
    W2_CHUNK = 256          # w2 columns per DMA chunk (2 h-tiles)
    NCH1 = FF // W1_CHUNK
    NCH2 = H // W2_CHUNK

    nc = bacc.Bacc("TRN2", target_bir_lowering=False, debug=False,
                   num_devices=N_CORES)

    # Weights/activations arrive pre-tiled to SBUF layout (host formats them)
    # so every DMA is a fully contiguous per-partition read.
    xgT_d = nc.dram_tensor("xgT", [P, H // P, S], BF16, kind="ExternalInput")
    w1_d = nc.dram_tensor("w1b", [E, NCH1, P, (H // P) * W1_CHUNK], BF16,
                          kind="ExternalInput")
    w2_d = nc.dram_tensor("w2b", [E, NCH2, P, (FF // P) * W2_CHUNK], BF16,
                          kind="ExternalInput")
    b1_d = nc.dram_tensor("b1f", [P, E, FF // P], F32, kind="ExternalInput")
    b2_d = nc.dram_tensor("b2f", [P, E, H // P], F32, kind="ExternalInput")
    cv_d = nc.dram_tensor("cv", [P, cvb[-1]], F32, kind="ExternalInput")
    ia_d = nc.dram_tensor("idxA", [P, T_PER_CORE // P], I32, kind="ExternalInput")
    ib_d = nc.dram_tensor("idxB", [P, T_PER_CORE // P], I32, kind="ExternalInput")
    out_d = nc.dram_tensor("out", [T_PER_CORE, H], F32, kind="ExternalOutput")
    ybuf = nc.dram_tensor("ybuf", [S, H], BF16)

    with tile.TileContext(nc) as tc:
        with (
            tc.tile_pool(name="const", bufs=1) as const_pool,
            tc.tile_pool(name="xg", bufs=1) as xg_pool,
            tc.tile_pool(name="w1", bufs=2) as w1_pool,
            tc.tile_pool(name="w2", bufs=2) as w2_pool,
            tc.tile_pool(name="hid", bufs=2) as hid_pool,
            tc.tile_pool(name="yt", bufs=2) as y_pool,
            tc.tile_pool(name="yrow", bufs=3) as yrow_pool,
            tc.tile_pool(name="gath", bufs=2) as g_pool,
            tc.tile_pool(name="ps1", bufs=3, space="PSUM") as ps1_pool,
            tc.tile_pool(name="ps2", bufs=3, space="PSUM") as ps2_pool,
            tc.tile_pool(name="pst", bufs=2, space="PSUM") as pst_pool,
        ):
            identity = const_pool.tile([P, P], BF16)
            make_identity(nc, identity[:])
            cv_sb = const_pool.tile([P, cvb[-1]], F32)
            nc.sync.dma_start(out=cv_sb[:], in_=cv_d[:])
            ia_sb = const_pool.tile([P, T_PER_CORE // P], I32)
            nc.sync.dma_start(out=ia_sb[:], in_=ia_d[:])
            ib_sb = const_pool.tile([P, T_PER_CORE // P], I32)
            nc.sync.dma_start(out=ib_sb[:], in_=ib_d[:])
            b1_sb = const_pool.tile([P, E, FF // P], F32)
            nc.sync.dma_start(out=b1_sb[:], in_=b1_d[:])
            b2_sb = const_pool.tile([P, E, H // P], F32)
            nc.sync.dma_start(out=b2_sb[:], in_=b2_d[:])

            xg_sb = xg_pool.tile([P, H // P, S], BF16)
            nc.sync.dma_start(out=xg_sb[:], in_=xgT_d[:])

            for e in range(E):
                cap = caps[e]
                seg = slice(bases[e], bases[e] + cap)
                # ---- mm1: hidden^T = gelu(w1[e].T @ xgT_seg + b1) ----
                hid = hid_pool.tile([P, FF // P, cap], BF16, tag="hid")
                for mc in range(NCH1):
                    w1t = w1_pool.tile([P, H // P, W1_CHUNK], BF16)
                    nc.sync.dma_start(
                        out=w1t[:],
                        in_=w1_d[e, mc].rearrange(
                            "p (ko m) -> p ko m", ko=H // P))
                    for mi in range(W1_CHUNK // P):
                        m = mc * (W1_CHUNK // P) + mi
                        ps = ps1_pool.tile([P, cap], F32)
                        for k in range(H // P):
                            nc.tensor.matmul(
                                ps[:],
                                lhsT=w1t[:, k, mi * P:(mi + 1) * P],
                                rhs=xg_sb[:, k, seg],
                                start=(k == 0),
                                stop=(k == H // P - 1),
                            )
                        nc.scalar.activation(
                            hid[:, m, :], ps[:], act_fn,
                            bias=b1_sb[:, e, m:m + 1])

                # ---- mm2: y^T = w2[e].T @ hidden^T + b2 ----
                y_sb = y_pool.tile([P, H // P, cap], BF16, tag="y")
                for hc in range(NCH2):
                    w2t = w2_pool.tile([P, FF // P, W2_CHUNK], BF16)
                    nc.sync.dma_start(
                        out=w2t[:],
                        in_=w2_d[e, hc].rearrange(
                            "p (ko n) -> p ko n", ko=FF // P))
                    for hi in range(W2_CHUNK // P):
                        h = hc * (W2_CHUNK // P) + hi
                        ps = ps2_pool.tile([P, cap], F32)
                        for k in range(FF // P):
                            nc.tensor.matmul(
                                ps[:],
                                lhsT=w2t[:, k, hi * P:(hi + 1) * P],
                                rhs=hid[:, k, :],
                                start=(k == 0),
                                stop=(k == FF // P - 1),
                            )
                        nc.vector.tensor_scalar_add(
                            y_sb[:, h, :], ps[:], b2_sb[:, e, h:h + 1])

                # ---- transpose to token-major, scale by combine, store ----
                for j in range(nsubs[e]):
                    w = min(P, cap - j * P)
                    yrow = yrow_pool.tile([P, H], BF16)
                    for h in range(H // P):
                        pt = pst_pool.tile([P, P], BF16)
                        nc.tensor.transpose(
                            pt[:w, :], y_sb[:, h, j * P:j * P + w], identity[:])
                        nc.vector.tensor_tensor(
                            out=yrow[:w, h * P:(h + 1) * P],
                            in0=pt[:w, :],
                            in1=cv_sb[:w, cvb[e] + j:cvb[e] + j + 1]
                                .to_broadcast([w, P]),
                            op=mybir.AluOpType.mult)
                    base = bases[e] + j * P
                    nc.sync.dma_start(out=ybuf[base:base + w, :], in_=yrow[:w, :])

            # ---- combine: out[t] = ybuf[slotA[t]] + ybuf[slotB[t]] ----
            for jt in range(T_PER_CORE // P):
                gA = g_pool.tile([P, H], BF16, tag="gA")
                gB = g_pool.tile([P, H], BF16, tag="gB")
                ot = g_pool.tile([P, H], F32, tag="ot")
                nc.gpsimd.indirect_dma_start(
                    out=gA[:], out_offset=None, in_=ybuf[:],
                    in_offset=IndirectOffsetOnAxis(ap=ia_sb[:, jt:jt + 1], axis=0))
                nc.gpsimd.indirect_dma_start(
                    out=gB[:], out_offset=None, in_=ybuf[:],
                    in_offset=IndirectOffsetOnAxis(ap=ib_sb[:, jt:jt + 1], axis=0))
                nc.vector.tensor_tensor(out=ot[:], in0=gA[:], in1=gB[:],
                                        op=mybir.AluOpType.add)
                nc.sync.dma_start(out=out_d[jt * P:(jt + 1) * P, :], in_=ot[:])

    nc.compile()
    return nc


# ----------------------------------------------------------------------------
# Entry point
# ----------------------------------------------------------------------------

def prepare(x, router_w, router_b, w1, b1, w2, b2):
    """Host-side sharding: returns (nc, in_maps, per-core token lists)."""
    x_flat = np.ascontiguousarray(np.asarray(x, np.float32).reshape(T, H))
    e1, e2, c1, c2 = _route(x_flat, np.asarray(router_w), np.asarray(router_b))
    assign, caps = _assign_tokens(e1, e2)

    x_flat_bf = x_flat.astype(NP_BF16)
    W1_CHUNK, W2_CHUNK = 512, 256
    # pre-tile weights to the SBUF slab layout: [e, chunk, p, ko*chunk_cols]
    w1b = np.ascontiguousarray(
        np.asarray(w1, np.float32).astype(NP_BF16)
        .reshape(E, H // P, P, FF // W1_CHUNK, W1_CHUNK)
        .transpose(0, 3, 2, 1, 4)
        .reshape(E, FF // W1_CHUNK, P, (H // P) * W1_CHUNK))
    w2b = np.ascontiguousarray(
        np.asarray(w2, np.float32).astype(NP_BF16)
        .reshape(E, FF // P, P, H // W2_CHUNK, W2_CHUNK)
        .transpose(0, 3, 2, 1, 4)
        .reshape(E, H // W2_CHUNK, P, (FF // P) * W2_CHUNK))
    b1f = np.ascontiguousarray(
        np.asarray(b1, np.float32).reshape(E, FF // P, P).transpose(2, 0, 1))
    b2f = np.ascontiguousarray(
        np.asarray(b2, np.float32).reshape(E, H // P, P).transpose(2, 0, 1))

    in_maps = []
    token_lists = []
    for c in range(N_CORES):
        cd = _build_core_inputs(x_flat_bf, e1, e2, c1, c2, assign, caps, c)
        in_maps.append(dict(xgT=cd["xgT"], w1b=w1b, w2b=w2b, b1f=b1f, b2f=b2f,
                            cv=cd["cv"], idxA=cd["idxA"], idxB=cd["idxB"]))
        token_lists.append(cd["tokens"])

    if caps not in _PROGRAM_CACHE:
        _PROGRAM_CACHE[caps] = build_program(caps)
    return _PROGRAM_CACHE[caps], in_maps, token_lists


def kernel(x, router_w, router_b, w1, b1, w2, b2):
    nc, in_maps, token_lists = prepare(x, router_w, router_b, w1, b1, w2, b2)
    res = run_bass_kernel_spmd(nc, in_maps, core_ids=list(range(N_CORES)))
    out_full = np.empty((T, H), np.float32)
    for c in range(N_CORES):
        out_full[token_lists[c]] = res.results[c]["out"]
    return out_full.reshape(B, SEQ, H)



# revision 2
# speedup vs baseline: 1.0910x; 1.0910x over previous
"""MoE layer (nn_MoELayer_81630148428171) as a Trainium2 Bass kernel on 8 NeuronCores.

Strategy (expert parallelism, resident weights, transpose-free mm2):
  - Router runs on host (jax-cpu, bitwise-identical ops to the reference).
  - Core c owns expert c: its w1/w2 (16.8MB bf16) are loaded once into SBUF
    and stay resident; only the core's routed tokens stream through.
  - Host gathers each expert's tokens into feature-major bf16 columns
    xgT [H, cap] (cap = max per-expert count, padded to 128); per 512-token
    block the device computes
        hid^T = gelu(w1.T @ xgT_blk + b1)        (PE, bf16 in / f32 acc)
        y     = hid^T.T @ w2                      (PE, token-major output)
    then scales y rows by the token's combine weight (DVE, per-partition
    scalar) and DMAs f32 rows to HBM.  No PE transposes, no intra-core
    gather: each slot belongs to exactly one expert.
  - Host unshard: out[t] = Y[slot of top1(t)] + Y[slot of top2(t)]
    (+ c1*b2[e1] + c2*b2[e2], exact in f32).  No collectives.
"""

import numpy as np
import ml_dtypes

import concourse.bacc as bacc
import concourse.mybir as mybir
import concourse.tile as tile
from concourse.bass_utils import run_bass_kernel_spmd

# Problem shapes (hardcoded per contract).
B, SEQ, H = 4, 2048, 1024
T = B * SEQ
FF = 4 * H
E = 8
TOP_K = 2
N_CORES = 8
P = 128
BLK = 512                      # token slots per device block
W1C, W2C = 4, 2                # weight chunk counts (f-chunks / h-halves)

BF16 = mybir.dt.bfloat16
F32 = mybir.dt.float32
NP_BF16 = ml_dtypes.bfloat16

_PROGRAM_CACHE: dict[int, object] = {}


# ----------------------------------------------------------------------------
# Host-side routing + sharding
# ----------------------------------------------------------------------------

def _route(x_flat, router_w, router_b):
    """Top-2 routing with bitwise-identical math to the jax reference."""
    try:
        import jax
        import jax.numpy as jnp

        cpu = jax.devices("cpu")[0]

        def f(xf, w, b):
            logits = xf @ w + b
            probs = jax.nn.softmax(logits, axis=-1)
            top_values, top_indices = jax.lax.top_k(probs, TOP_K)
            top_values = top_values / jnp.sum(top_values, axis=-1,
                                              keepdims=True)
            return top_values, top_indices

        with jax.default_device(cpu):
            tv, ti = jax.jit(f)(
                jnp.asarray(x_flat), jnp.asarray(router_w),
                jnp.asarray(router_b))
        tv = np.asarray(tv)
        ti = np.asarray(ti)
    except Exception:
        logits = x_flat @ router_w + router_b
        p = np.exp(logits - logits.max(-1, keepdims=True))
        p /= p.sum(-1, keepdims=True)
        ti = np.argsort(-p, axis=-1, kind="stable")[:, :TOP_K]
        tv = np.take_along_axis(p, ti, axis=-1)
        tv = tv / tv.sum(-1, keepdims=True)
    return (
        ti[:, 0].astype(np.int64),
        ti[:, 1].astype(np.int64),
        tv[:, 0].astype(np.float32),
        tv[:, 1].astype(np.float32),
    )


# ----------------------------------------------------------------------------
# Device program
# ----------------------------------------------------------------------------

def build_program(cap, act_fn=None):
    """SPMD program; each core runs one expert over `cap` token slots
    (cap a multiple of 128; per-core differences come only via inputs)."""
    if act_fn is None:
        act_fn = mybir.ActivationFunctionType.Gelu
    assert cap % P == 0
    nblk = (cap + BLK - 1) // BLK

    nc = bacc.Bacc("TRN2", target_bir_lowering=False, debug=False,
                   num_devices=N_CORES)

    # All arrays arrive pre-tiled to SBUF layout (host formats them) so every
    # DMA is a fully contiguous per-partition read.
    xg_d = nc.dram_tensor("xg", [P, H // P, cap], BF16, kind="ExternalInput")
    w1_d = nc.dram_tensor("w1b", [W1C, P, H // P, FF // W1C], BF16,
                          kind="ExternalInput")
    w2_d = nc.dram_tensor("w2b", [W2C, P, FF // P, H // W2C], BF16,
                          kind="ExternalInput")
    b1_d = nc.dram_tensor("b1f", [P, FF // P], F32, kind="ExternalInput")
    cv_d = nc.dram_tensor("cv", [P, cap // P], F32, kind="ExternalInput")
    y_d = nc.dram_tensor("y", [cap, H], F32, kind="ExternalOutput")

    with tile.TileContext(nc) as tc:
        with (
            tc.tile_pool(name="const", bufs=1) as const_pool,
            tc.tile_pool(name="w1", bufs=W1C) as w1_pool,
            tc.tile_pool(name="w2", bufs=W2C) as w2_pool,
            tc.tile_pool(name="xg", bufs=2) as xg_pool,
            tc.tile_pool(name="hid", bufs=1) as hid_pool,
            tc.tile_pool(name="yt", bufs=3) as y_pool,
            tc.tile_pool(name="ps1", bufs=3, space="PSUM") as ps1_pool,
            tc.tile_pool(name="ps2", bufs=3, space="PSUM") as ps2_pool,
        ):
            b1_sb = const_pool.tile([P, FF // P], F32)
            nc.sync.dma_start(out=b1_sb[:], in_=b1_d[:])
            cv_sb = const_pool.tile([P, cap // P], F32)
            nc.sync.dma_start(out=cv_sb[:], in_=cv_d[:])

            # Resident weights, chunked so compute can start after chunk 0.
            w1c = []
            for mc in range(W1C):
                t = w1_pool.tile([P, H // P, FF // W1C], BF16, tag="w1")
                nc.sync.dma_start(out=t[:], in_=w1_d[mc])
                w1c.append(t)
            w2c = []
            for hc in range(W2C):
                t = w2_pool.tile([P, FF // P, H // W2C], BF16, tag="w2")
                nc.sync.dma_start(out=t[:], in_=w2_d[hc])
                w2c.append(t)

            m_per_chunk = (FF // W1C) // P          # 8 m-tiles per w1 chunk
            for b in range(nblk):
                b0 = b * BLK
                w = min(BLK, cap - b0)
                xg_t = xg_pool.tile([P, H // P, w], BF16, tag="xg")
                nc.sync.dma_start(out=xg_t[:], in_=xg_d[:, :, b0:b0 + w])

                # ---- mm1: hid^T = gelu(w1.T @ xg_blk + b1) ----
                hid = hid_pool.tile([P, FF // P, w], BF16, tag="hid")
                for m in range(FF // P):
                    mc, mi = divmod(m, m_per_chunk)
                    ps = ps1_pool.tile([P, w], F32, tag="ps1")
                    for k in range(H // P):
                        nc.tensor.matmul(
                            ps[:],
                            lhsT=w1c[mc][:, k, mi * P:(mi + 1) * P],
                            rhs=xg_t[:, k, :],
                            start=(k == 0),
                            stop=(k == H // P - 1),
                        )
                    nc.scalar.activation(
                        hid[:, m, :], ps[:], act_fn,
                        bias=b1_sb[:, m:m + 1])

                # ---- mm2 (token-major): y[tok, h] = hid^T.T @ w2 ----
                for jt in range(w // P):
                    for hc in range(W2C):
                        ps = ps2_pool.tile([P, H // W2C], F32, tag="ps2")
                        for k in range(FF // P):
                            nc.tensor.matmul(
                                ps[:],
                                lhsT=hid[:, k, jt * P:(jt + 1) * P],
                                rhs=w2c[hc][:, k, :],
                                start=(k == 0),
                                stop=(k == FF // P - 1),
                            )
                        yt = y_pool.tile([P, H // W2C], F32, tag="yt")
                        nc.vector.tensor_scalar_mul(
                            yt[:], ps[:],
                            cv_sb[:, b0 // P + jt:b0 // P + jt + 1])
                        nc.sync.dma_start(
                            out=y_d[b0 + jt * P:b0 + (jt + 1) * P,
                                    hc * (H // W2C):(hc + 1) * (H // W2C)],
                            in_=yt[:])

    nc.compile()
    return nc


# ----------------------------------------------------------------------------
# Entry point
# ----------------------------------------------------------------------------

def prepare(x, router_w, router_b, w1, b1, w2, b2):
    """Host-side sharding: returns (nc, in_maps, combine_info)."""
    x_flat = np.ascontiguousarray(np.asarray(x, np.float32).reshape(T, H))
    e1, e2, c1, c2 = _route(x_flat, np.asarray(router_w), np.asarray(router_b))
    w1 = np.asarray(w1, np.float32)
    b1 = np.asarray(b1, np.float32)
    w2 = np.asarray(w2, np.float32)
    b2 = np.asarray(b2, np.float32)

    # Slot assignment: expert e's tokens in token order, slot = position.
    ee = np.concatenate([e1, e2])                      # [2T] expert of entry
    cc = np.concatenate([c1, c2])                      # [2T] combine weight
    tt = np.concatenate([np.arange(T), np.arange(T)])  # [2T] token of entry
    order = np.argsort(ee, kind="stable")              # group entries by expert
    counts = np.bincount(ee, minlength=E)
    cap = max(P, int(np.ceil(counts.max() / P)) * P)
    starts = np.zeros(E + 1, np.int64)
    starts[1:] = np.cumsum(counts)

    # Global slot id of each entry (expert*cap + within-expert position).
    slot_of_entry = np.empty(2 * T, np.int64)
    within = np.arange(2 * T) - starts[ee[order]]
    slot_of_entry[order] = ee[order] * cap + within
    idx1, idx2 = slot_of_entry[:T], slot_of_entry[T:]

    x_flat_bf = x_flat.astype(NP_BF16)
    in_maps = []
    for c in range(N_CORES):
        sel = tt[order[starts[c]:starts[c + 1]]]       # tokens routed here
        n = len(sel)
        xgT = np.zeros((H, cap), NP_BF16)
        xgT[:, :n] = x_flat_bf[sel].T
        xg = np.ascontiguousarray(
            xgT.reshape(H // P, P, cap).transpose(1, 0, 2))
        cvals = np.zeros(cap, np.float32)
        cvals[:n] = cc[order[starts[c]:starts[c + 1]]]
        cv = np.ascontiguousarray(cvals.reshape(cap // P, P).T)

        w1b = np.ascontiguousarray(
            w1[c].astype(NP_BF16).reshape(H // P, P, W1C, FF // W1C)
            .transpose(2, 1, 0, 3))
        w2b = np.ascontiguousarray(
            w2[c].astype(NP_BF16).reshape(FF // P, P, W2C, H // W2C)
            .transpose(2, 1, 0, 3))
        b1f = np.ascontiguousarray(b1[c].reshape(FF // P, P).T)
        in_maps.append(dict(xg=xg, w1b=w1b, w2b=w2b, b1f=b1f, cv=cv))

    # Exact f32 bias contribution (sum of combine weights is 1 per token).
    bias_add = c1[:, None] * b2[e1] + c2[:, None] * b2[e2]
    combine_info = (cap, idx1, idx2, bias_add)

    if cap not in _PROGRAM_CACHE:
        _PROGRAM_CACHE[cap] = build_program(cap)
    return _PROGRAM_CACHE[cap], in_maps, combine_info


def kernel(x, router_w, router_b, w1, b1, w2, b2):
    nc, in_maps, (cap, idx1, idx2, bias_add) = prepare(
        x, router_w, router_b, w1, b1, w2, b2)
    res = run_bass_kernel_spmd(nc, in_maps, core_ids=list(range(N_CORES)))
    Y = np.concatenate([res.results[c]["y"] for c in range(N_CORES)], axis=0)
    out_full = Y[idx1] + Y[idx2] + bias_add
    return out_full.reshape(B, SEQ, H).astype(np.float32)


# revision 5
# speedup vs baseline: 4.2221x; 3.8700x over previous
"""MoE layer (nn_MoELayer_81630148428171) as a Trainium2 Bass kernel on 8 NeuronCores.

Strategy (balanced expert parallelism, resident weights, transpose-free mm2):
  - Router runs on host (jax-cpu, bitwise-identical ops to the reference).
  - Each core runs two single-expert segments of fixed sizes (capA, capB):
    a 128-aligned two-slots-per-core packing of the per-expert token counts
    chosen so capA+capB is minimal (hot experts may span 3 slots).  This
    beats one-expert-per-core, whose per-core cost is the *max* expert
    count rounded up to 128.
  - Host gathers each slot's tokens into feature-major bf16 columns; per
    512-token block the device computes
        hid^T = gelu(w1.T @ xgT_blk + b1)        (PE, bf16 in / f32 acc)
        y     = hid^T.T @ w2                      (PE, token-major output)
    then scales y rows by the token's combine weight (DVE, per-partition
    scalar) and DMAs f32 rows to HBM.  Segment weights (16.8MB bf16) are
    SBUF-resident during their segment; the next segment's stream in under
    the current segment's compute.  No PE transposes, no intra-core gather.
  - Host unshard: out[t] = Y[slot of top1(t)] + Y[slot of top2(t)]
    (+ c1*b2[e1] + c2*b2[e2], exact in f32).  No collectives.
"""

import numpy as np
import ml_dtypes

import concourse.bacc as bacc
import concourse.mybir as mybir
import concourse.tile as tile
from concourse.bass_utils import run_bass_kernel_spmd

# Problem shapes (hardcoded per contract).
B, SEQ, H = 4, 2048, 1024
T = B * SEQ
FF = 4 * H
E = 8
TOP_K = 2
N_CORES = 8
P = 128
BLK = 512                      # token slots per device block
W1C, W2C = 4, 2                # weight chunk counts (f-chunks / h-halves)

BF16 = mybir.dt.bfloat16
F32 = mybir.dt.float32
NP_BF16 = ml_dtypes.bfloat16

_PROGRAM_CACHE: dict[tuple, object] = {}
_PROGRAM_KEY: tuple = None


# ----------------------------------------------------------------------------
# Host-side routing + sharding
# ----------------------------------------------------------------------------

def _route(x_flat, router_w, router_b):
    """Top-2 routing with bitwise-identical math to the jax reference."""
    try:
        import jax
        import jax.numpy as jnp

        cpu = jax.devices("cpu")[0]

        def f(xf, w, b):
            logits = xf @ w + b
            probs = jax.nn.softmax(logits, axis=-1)
            top_values, top_indices = jax.lax.top_k(probs, TOP_K)
            top_values = top_values / jnp.sum(top_values, axis=-1,
                                              keepdims=True)
            return top_values, top_indices

        with jax.default_device(cpu):
            tv, ti = jax.jit(f)(
                jnp.asarray(x_flat), jnp.asarray(router_w),
                jnp.asarray(router_b))
        tv = np.asarray(tv)
        ti = np.asarray(ti)
    except Exception:
        logits = x_flat @ router_w + router_b
        p = np.exp(logits - logits.max(-1, keepdims=True))
        p /= p.sum(-1, keepdims=True)
        ti = np.argsort(-p, axis=-1, kind="stable")[:, :TOP_K]
        tv = np.take_along_axis(p, ti, axis=-1)
        tv = tv / tv.sum(-1, keepdims=True)
    return (
        ti[:, 0].astype(np.int64),
        ti[:, 1].astype(np.int64),
        tv[:, 0].astype(np.float32),
        tv[:, 1].astype(np.float32),
    )


def _pack(counts):
    """Pack per-expert token counts into 8 cores x 2 single-expert slots of
    sizes (capA, capB), 128-aligned, minimizing capA+capB (device time is
    proportional to it).  Experts may split across slots/cores.

    Returns (capA, capB, slotsA, slotsB) where slots*[core] is
    (expert, start, size) — `start` an offset into the expert's token list —
    or size 0 for an unused slot."""
    counts = np.asarray(counts, np.int64)
    total = int(counts.sum())
    s_min = max(P, -(-(-(-total // N_CORES)) // P) * P)  # ceil twice
    s_min = max(P, ((total + N_CORES - 1) // N_CORES + P - 1) // P * P)
    s_max = max(P, (int(counts.max()) + P - 1) // P * P)

    def ffd(cap_list):
        """First-fit-decreasing with splitting; best-fit for remainders."""
        free = sorted(cap_list, reverse=True)      # slot sizes
        slots = []                                 # (size, expert, start)
        for e in np.argsort(-counts):
            rem = int(counts[e])
            start = 0
            while rem > 0:
                if not free:
                    return None
                fit = [c for c in free if c >= rem]
                c = min(fit) if fit else max(free)
                free.remove(c)
                piece = min(rem, c)
                slots.append((c, int(e), start, piece))
                start += piece
                rem -= piece
        for c in free:
            slots.append((c, 0, 0, 0))
        return slots

    for S in range(s_min, s_max + 2 * P, P):
        caps = []
        for capA in range((S + 1) // 2 // P * P, S + 1, P):
            capB = S - capA
            if capA < capB or capB < 0:
                continue
            caps.append((capA, capB))
        for capA, capB in sorted(caps):
            sl = ffd([capA] * N_CORES + [capB] * N_CORES if capB else
                     [capA] * N_CORES)
            if sl is None:
                continue
            slotsA = [s for s in sl if s[0] == capA]
            slotsB = [s for s in sl if s[0] == capB and capB != capA]
            if capB == capA:
                slotsA, slotsB = sl[:N_CORES], sl[N_CORES:]
            if capB == 0:
                slotsB = [(0, 0, 0, 0)] * N_CORES
            if len(slotsA) != N_CORES or len(slotsB) != N_CORES:
                continue
            A = [(e, st, sz) for (_, e, st, sz) in slotsA]
            Bs = [(e, st, sz) for (_, e, st, sz) in slotsB]
            return capA, capB, A, Bs
    # unreachable: S = s_max with capB=0 always fits
    raise AssertionError("packing failed")


# ----------------------------------------------------------------------------
# Device program
# ----------------------------------------------------------------------------

def build_program(key, act_fn=None, reps=1):
    """SPMD program; each core runs one single-expert segment of capA slots
    then one of capB slots (compile-time sizes; per-core differences come
    only via inputs).  nrealA/nrealB are the largest real (unpadded) piece
    sizes: each segment's tail block only computes that many columns.

    `reps` unrolls the whole body (including weight loads) that many times
    inside one NEFF — used only for timing, so the per-exec cost can be
    derived without per-dispatch overhead."""
    capA, nrealA, capB, nrealB = key
    if act_fn is None:
        act_fn = mybir.ActivationFunctionType.Gelu
    assert capA % P == 0 and capB % P == 0
    segs = [(0, capA, nrealA)] + ([(capA, capB, nrealB)] if capB else [])
    cap = capA + capB
    n_seg = len(segs)

    nc = bacc.Bacc("TRN2", target_bir_lowering=False, debug=False,
                   num_devices=N_CORES)

    # All arrays arrive pre-tiled to SBUF layout (host formats them) so every
    # DMA is a fully contiguous per-partition read.
    xg_d = nc.dram_tensor("xg", [P, H // P, cap], BF16, kind="ExternalInput")
    w1_d = nc.dram_tensor("w1b", [n_seg, W1C, P, H // P, FF // W1C], BF16,
                          kind="ExternalInput")
    w2_d = nc.dram_tensor("w2b", [n_seg, W2C, P, FF // P, H // W2C], BF16,
                          kind="ExternalInput")
    b1_d = nc.dram_tensor("b1f", [n_seg, P, FF // P], F32,
                          kind="ExternalInput")
    cv_d = nc.dram_tensor("cv", [P, cap // P], F32, kind="ExternalInput")
    y_d = nc.dram_tensor("y", [cap, H], F32, kind="ExternalOutput")

    with tile.TileContext(nc) as tc:
        with (
            tc.tile_pool(name="const", bufs=2) as const_pool,
            tc.tile_pool(name="w1", bufs=W1C) as w1_pool,
            tc.tile_pool(name="w2", bufs=W2C) as w2_pool,
            tc.tile_pool(name="xg", bufs=2) as xg_pool,
            tc.tile_pool(name="hid", bufs=1) as hid_pool,
            tc.tile_pool(name="yt", bufs=3) as y_pool,
            tc.tile_pool(name="ps1", bufs=3, space="PSUM") as ps1_pool,
            tc.tile_pool(name="ps2", bufs=3, space="PSUM") as ps2_pool,
        ):
          for rep in range(reps):
            # first xg block first: compute starts as soon as it and the
            # first w1 chunk land; everything else hides under compute.
            xg0 = xg_pool.tile([P, H // P, min(BLK, capA)], BF16, tag="xg")
            nc.sync.dma_start(out=xg0[:], in_=xg_d[:, :, 0:min(BLK, capA)])
            cv_sb = const_pool.tile([P, cap // P], F32, tag="cv")
            nc.sync.dma_start(out=cv_sb[:], in_=cv_d[:])

            m_per_chunk = (FF // W1C) // P          # 8 m-tiles per w1 chunk
            for s, (sbase, scap, snreal) in enumerate(segs):
                # this segment's weights (SBUF-resident while it runs; the
                # pool slots make the next segment's loads wait for the
                # current segment's last reads, so they stream in behind)
                w1c = []
                for mc in range(W1C):
                    t = w1_pool.tile([P, H // P, FF // W1C], BF16, tag="w1")
                    nc.sync.dma_start(out=t[:], in_=w1_d[s, mc])
                    w1c.append(t)
                w2c = []
                for hc in range(W2C):
                    t = w2_pool.tile([P, FF // P, H // W2C], BF16, tag="w2")
                    nc.sync.dma_start(out=t[:], in_=w2_d[s, hc])
                    w2c.append(t)
                b1_sb = const_pool.tile([P, FF // P], F32, tag="b1")
                nc.sync.dma_start(out=b1_sb[:], in_=b1_d[s])

                nblk = (scap + BLK - 1) // BLK
                for b in range(nblk):
                    b0 = b * BLK
                    w = min(BLK, scap - b0)
                    we = min(w, max(snreal - b0, 0))     # real columns
                    g0 = sbase + b0                      # global slot base
                    if s == 0 and b == 0:
                        xg_t = xg0
                    else:
                        xg_t = xg_pool.tile([P, H // P, w], BF16, tag="xg")
                        nc.sync.dma_start(out=xg_t[:],
                                          in_=xg_d[:, :, g0:g0 + w])

                    # ---- mm1: hid^T = gelu(w1.T @ xg_blk + b1) ----
                    hid = hid_pool.tile([P, FF // P, w], BF16, tag="hid")
                    if we < w:
                        nc.vector.memset(hid[:, :, we:], 0.0)
                    for m in range(FF // P):
                        if we == 0:
                            break
                        mc, mi = divmod(m, m_per_chunk)
                        ps = ps1_pool.tile([P, we], F32, tag="ps1")
                        for k in range(H // P):
                            nc.tensor.matmul(
                                ps[:],
                                lhsT=w1c[mc][:, k, mi * P:(mi + 1) * P],
                                rhs=xg_t[:, k, :we],
                                start=(k == 0),
                                stop=(k == H // P - 1),
                            )
                        nc.scalar.activation(
                            hid[:, m, :we], ps[:], act_fn,
                            bias=b1_sb[:, m:m + 1])

                    # ---- mm2 (token-major): y[tok, h] = hid^T.T @ w2 ----
                    for jt in range(w // P):
                        for hc in range(W2C):
                            ps = ps2_pool.tile([P, H // W2C], F32, tag="ps2")
                            for k in range(FF // P):
                                nc.tensor.matmul(
                                    ps[:],
                                    lhsT=hid[:, k, jt * P:(jt + 1) * P],
                                    rhs=w2c[hc][:, k, :],
                                    start=(k == 0),
                                    stop=(k == FF // P - 1),
                                )
                            yt = y_pool.tile([P, H // W2C], F32, tag="yt")
                            nc.vector.tensor_scalar_mul(
                                yt[:], ps[:],
                                cv_sb[:, g0 // P + jt:g0 // P + jt + 1])
                            nc.sync.dma_start(
                                out=y_d[g0 + jt * P:g0 + (jt + 1) * P,
                                        hc * (H // W2C):(hc + 1) * (H // W2C)],
                                in_=yt[:])

    nc.compile()
    return nc


# ----------------------------------------------------------------------------
# Entry point
# ----------------------------------------------------------------------------

def prepare(x, router_w, router_b, w1, b1, w2, b2):
    """Host-side sharding: returns (nc, in_maps, combine_info)."""
    x_flat = np.ascontiguousarray(np.asarray(x, np.float32).reshape(T, H))
    e1, e2, c1, c2 = _route(x_flat, np.asarray(router_w), np.asarray(router_b))
    w1 = np.asarray(w1, np.float32)
    b1 = np.asarray(b1, np.float32)
    w2 = np.asarray(w2, np.float32)
    b2 = np.asarray(b2, np.float32)

    # Group routed entries by expert (token order within an expert).
    ee = np.concatenate([e1, e2])                      # [2T] expert of entry
    cc = np.concatenate([c1, c2])                      # [2T] combine weight
    tt = np.concatenate([np.arange(T), np.arange(T)])  # [2T] token of entry
    order = np.argsort(ee, kind="stable")              # entries by expert
    counts = np.bincount(ee, minlength=E)
    starts = np.zeros(E + 1, np.int64)
    starts[1:] = np.cumsum(counts)

    capA, capB, slotsA, slotsB = _pack(counts)
    cap = capA + capB
    nrealA = max(sz for (_, _, sz) in slotsA)
    nrealB = max(sz for (_, _, sz) in slotsB) if capB else 0

    # Global Y row of each entry: core*cap + seg base + offset in piece.
    slot_of_entry = np.empty(2 * T, np.int64)
    x_flat_bf = x_flat.astype(NP_BF16)
    in_maps = []
    for c in range(N_CORES):
        xgT = np.zeros((H, cap), NP_BF16)
        cvals = np.zeros(cap, np.float32)
        w1b = np.zeros((2 if capB else 1, W1C, P, H // P, FF // W1C), NP_BF16)
        w2b = np.zeros((2 if capB else 1, W2C, P, FF // P, H // W2C), NP_BF16)
        b1f = np.zeros((2 if capB else 1, P, FF // P), np.float32)
        pieces = [(0, slotsA[c])] + ([(capA, slotsB[c])] if capB else [])
        for sbase, (e, st, sz) in pieces:
            if sz > 0:
                ent = order[starts[e] + st:starts[e] + st + sz]
                xgT[:, sbase:sbase + sz] = x_flat_bf[tt[ent]].T
                cvals[sbase:sbase + sz] = cc[ent]
                slot_of_entry[ent] = c * cap + sbase + np.arange(sz)
            s = 0 if sbase == 0 else 1
            w1b[s] = (w1[e].astype(NP_BF16)
                      .reshape(H // P, P, W1C, FF // W1C).transpose(2, 1, 0, 3))
            w2b[s] = (w2[e].astype(NP_BF16)
                      .reshape(FF // P, P, W2C, H // W2C).transpose(2, 1, 0, 3))
            b1f[s] = b1[e].reshape(FF // P, P).T
        xg = np.ascontiguousarray(
            xgT.reshape(H // P, P, cap).transpose(1, 0, 2))
        cv = np.ascontiguousarray(cvals.reshape(cap // P, P).T)
        in_maps.append(dict(xg=xg, w1b=np.ascontiguousarray(w1b),
                            w2b=np.ascontiguousarray(w2b), b1f=b1f, cv=cv))

    idx1, idx2 = slot_of_entry[:T], slot_of_entry[T:]
    # Exact f32 bias contribution (sum of combine weights is 1 per token).
    bias_add = c1[:, None] * b2[e1] + c2[:, None] * b2[e2]
    combine_info = (cap, idx1, idx2, bias_add)

    key = (capA, nrealA, capB, nrealB)
    global _PROGRAM_KEY
    _PROGRAM_KEY = key
    if key not in _PROGRAM_CACHE:
        _PROGRAM_CACHE[key] = build_program(key)
    return _PROGRAM_CACHE[key], in_maps, combine_info


def kernel(x, router_w, router_b, w1, b1, w2, b2):
    nc, in_maps, (cap, idx1, idx2, bias_add) = prepare(
        x, router_w, router_b, w1, b1, w2, b2)
    res = run_bass_kernel_spmd(nc, in_maps, core_ids=list(range(N_CORES)))
    Y = np.concatenate([res.results[c]["y"] for c in range(N_CORES)], axis=0)
    out_full = Y[idx1] + Y[idx2] + bias_add
    return out_full.reshape(B, SEQ, H).astype(np.float32)


# revision 7
# speedup vs baseline: 4.6611x; 1.1040x over previous
"""MoE layer (nn_MoELayer_81630148428171) as a Trainium2 Bass kernel on 8 NeuronCores.

Strategy (balanced expert parallelism, resident weights, transpose-free mm2):
  - Router runs on host (jax-cpu, bitwise-identical ops to the reference).
  - Each core runs two single-expert segments of fixed sizes (capA, capB):
    a 128-aligned two-slots-per-core packing of the per-expert token counts
    chosen so capA+capB is minimal (hot experts may span 3 slots).  This
    beats one-expert-per-core, whose per-core cost is the *max* expert
    count rounded up to 128.
  - Host gathers each slot's tokens into feature-major bf16 columns; per
    512-token block the device computes
        hid^T = gelu(w1.T @ xgT_blk + b1)        (PE, bf16 in / f32 acc)
        y     = hid^T.T @ w2                      (PE, token-major output)
    then scales y rows by the token's combine weight (DVE, per-partition
    scalar) and DMAs f32 rows to HBM.  Segment weights (16.8MB bf16) are
    SBUF-resident during their segment; the next segment's stream in under
    the current segment's compute.  No PE transposes, no intra-core gather.
  - Host unshard: out[t] = Y[slot of top1(t)] + Y[slot of top2(t)]
    (+ c1*b2[e1] + c2*b2[e2], exact in f32).  No collectives.
"""

import numpy as np
import ml_dtypes

import concourse.bacc as bacc
import concourse.mybir as mybir
import concourse.tile as tile
from concourse.bass_utils import run_bass_kernel_spmd

# Problem shapes (hardcoded per contract).
B, SEQ, H = 4, 2048, 1024
T = B * SEQ
FF = 4 * H
E = 8
TOP_K = 2
N_CORES = 8
P = 128
BLK = 512                      # token slots per device block
W1C, W2C = 4, 2                # weight chunk counts (f-chunks / h-halves)

BF16 = mybir.dt.bfloat16
F32 = mybir.dt.float32
NP_BF16 = ml_dtypes.bfloat16

_PROGRAM_CACHE: dict[tuple, object] = {}
_PROGRAM_KEY: tuple = None


# ----------------------------------------------------------------------------
# Host-side routing + sharding
# ----------------------------------------------------------------------------

def _route(x_flat, router_w, router_b):
    """Top-2 routing with bitwise-identical math to the jax reference."""
    try:
        import jax
        import jax.numpy as jnp

        cpu = jax.devices("cpu")[0]

        def f(xf, w, b):
            logits = xf @ w + b
            probs = jax.nn.softmax(logits, axis=-1)
            top_values, top_indices = jax.lax.top_k(probs, TOP_K)
            top_values = top_values / jnp.sum(top_values, axis=-1,
                                              keepdims=True)
            return top_values, top_indices

        with jax.default_device(cpu):
            tv, ti = jax.jit(f)(
                jnp.asarray(x_flat), jnp.asarray(router_w),
                jnp.asarray(router_b))
        tv = np.asarray(tv)
        ti = np.asarray(ti)
    except Exception:
        logits = x_flat @ router_w + router_b
        p = np.exp(logits - logits.max(-1, keepdims=True))
        p /= p.sum(-1, keepdims=True)
        ti = np.argsort(-p, axis=-1, kind="stable")[:, :TOP_K]
        tv = np.take_along_axis(p, ti, axis=-1)
        tv = tv / tv.sum(-1, keepdims=True)
    return (
        ti[:, 0].astype(np.int64),
        ti[:, 1].astype(np.int64),
        tv[:, 0].astype(np.float32),
        tv[:, 1].astype(np.float32),
    )


def _pack(counts):
    """Pack per-expert token counts into 8 cores x 2 single-expert slots of
    sizes (capA, capB), 128-aligned, minimizing capA+capB (device time is
    proportional to it).  Experts may split across slots/cores.

    Returns (capA, capB, slotsA, slotsB) where slots*[core] is
    (expert, start, size) — `start` an offset into the expert's token list —
    or size 0 for an unused slot."""
    counts = np.asarray(counts, np.int64)
    total = int(counts.sum())
    s_min = max(P, -(-(-(-total // N_CORES)) // P) * P)  # ceil twice
    s_min = max(P, ((total + N_CORES - 1) // N_CORES + P - 1) // P * P)
    s_max = max(P, (int(counts.max()) + P - 1) // P * P)

    def ffd(cap_list):
        """First-fit-decreasing with splitting; best-fit for remainders."""
        free = sorted(cap_list, reverse=True)      # slot sizes
        slots = []                                 # (size, expert, start)
        for e in np.argsort(-counts):
            rem = int(counts[e])
            start = 0
            while rem > 0:
                if not free:
                    return None
                fit = [c for c in free if c >= rem]
                c = min(fit) if fit else max(free)
                free.remove(c)
                piece = min(rem, c)
                slots.append((c, int(e), start, piece))
                start += piece
                rem -= piece
        for c in free:
            slots.append((c, 0, 0, 0))
        return rebalance(slots)

    def rebalance(slots):
        """Minimax re-split of each expert across its assigned slots: the
        per-segment mm1 width is the *largest* piece in that segment, so
        equalizing pieces (subject to slot caps) trims real compute."""
        by_e = {}
        for i, (c, e, st, sz) in enumerate(slots):
            if sz > 0:
                by_e.setdefault(e, []).append(i)
        out = list(slots)
        for e, idxs in by_e.items():
            idxs.sort(key=lambda i: slots[i][0])   # caps ascending
            rem = sum(slots[i][3] for i in idxs)
            caps = [slots[i][0] for i in idxs]
            start = 0
            for j, i in enumerate(idxs):
                left = len(idxs) - j - 1
                lo = rem - sum(caps[j + 1:])
                piece = min(caps[j], max(-(-rem // (left + 1)), lo, 0))
                out[i] = (slots[i][0], e, start, piece)
                start += piece
                rem -= piece
            assert rem == 0
        return out

    def cost(res):
        """Modeled PE cycles: mm1 scales with real widths, mm2 with the
        128-token-tile counts they occupy."""
        capA, capB, A, Bs = res
        nA = max(sz for (_, _, sz) in A)
        nB = max((sz for (_, _, sz) in Bs), default=0)
        return (256 * (nA + nB)
                + 256 * P * (-(-nA // P) + -(-nB // P)))

    def halved():
        """Each of the E//2 largest experts halved across two A-slots, the
        rest halved across two B-slots (needs E == N_CORES, 2 slots/core).
        Optimal when feasible: nA = max/2 over group A, nB over group B."""
        if E != N_CORES:
            return None
        idx = np.argsort(-counts)
        GA, GB = idx[:E // 2], idx[E // 2:]
        halfA = [(int(counts[e]) + 1) // 2 for e in GA]
        halfB = [(int(counts[e]) + 1) // 2 for e in GB]
        capA = max(P, (max(halfA) + P - 1) // P * P)
        capB = max(P, (max(halfB) + P - 1) // P * P)
        A, Bs = [], []
        for i in range(N_CORES):
            e = GA[i % (E // 2)]
            h = (int(counts[e]) + 1) // 2
            A.append((int(e), 0, h) if i < E // 2 else
                     (int(e), h, int(counts[e]) - h))
            e = GB[i % (E // 2)]
            h = (int(counts[e]) + 1) // 2
            Bs.append((int(e), 0, h) if i < E // 2 else
                      (int(e), h, int(counts[e]) - h))
        return capA, capB, A, Bs

    results = []
    for S in range(s_min, s_max + 2 * P, P):
        caps = []
        for capA in range((S + 1) // 2 // P * P, S + 1, P):
            capB = S - capA
            if capA < capB or capB < 0:
                continue
            caps.append((capA, capB))
        for capA, capB in sorted(caps):
            sl = ffd([capA] * N_CORES + [capB] * N_CORES if capB else
                     [capA] * N_CORES)
            if sl is None:
                continue
            slotsA = [s for s in sl if s[0] == capA]
            slotsB = [s for s in sl if s[0] == capB and capB != capA]
            if capB == capA:
                slotsA, slotsB = sl[:N_CORES], sl[N_CORES:]
            if capB == 0:
                slotsB = [(0, 0, 0, 0)] * N_CORES
            if len(slotsA) != N_CORES or len(slotsB) != N_CORES:
                continue
            A = [(e, st, sz) for (_, e, st, sz) in slotsA]
            Bs = [(e, st, sz) for (_, e, st, sz) in slotsB]
            results.append((capA, capB, A, Bs))
            break
        if results:
            break
    hv = halved()
    if hv is not None:
        results.append(hv)
    if not results:
        raise AssertionError("packing failed")
    return min(results, key=cost)


# ----------------------------------------------------------------------------
# Device program
# ----------------------------------------------------------------------------

def build_program(key, act_fn=None, reps=1):
    """SPMD program; each core runs one single-expert segment of capA slots
    then one of capB slots (compile-time sizes; per-core differences come
    only via inputs).  nrealA/nrealB are the largest real (unpadded) piece
    sizes: each segment's tail block only computes that many columns.

    `reps` unrolls the whole body (including weight loads) that many times
    inside one NEFF — used only for timing, so the per-exec cost can be
    derived without per-dispatch overhead."""
    capA, nrealA, capB, nrealB = key
    if act_fn is None:
        act_fn = mybir.ActivationFunctionType.Gelu
    assert capA % P == 0 and capB % P == 0
    segs = [(0, capA, nrealA)] + ([(capA, capB, nrealB)] if capB else [])
    cap = capA + capB
    n_seg = len(segs)

    nc = bacc.Bacc("TRN2", target_bir_lowering=False, debug=False,
                   num_devices=N_CORES)

    # All arrays arrive pre-tiled to SBUF layout (host formats them) so every
    # DMA is a fully contiguous per-partition read.
    xg_d = nc.dram_tensor("xg", [P, H // P, cap], BF16, kind="ExternalInput")
    w1_d = nc.dram_tensor("w1b", [n_seg, W1C, P, H // P, FF // W1C], BF16,
                          kind="ExternalInput")
    w2_d = nc.dram_tensor("w2b", [n_seg, W2C, P, FF // P, H // W2C], BF16,
                          kind="ExternalInput")
    b1_d = nc.dram_tensor("b1f", [n_seg, P, FF // P], F32,
                          kind="ExternalInput")
    cv_d = nc.dram_tensor("cv", [P, cap // P], F32, kind="ExternalInput")
    y_d = nc.dram_tensor("y", [cap, H], F32, kind="ExternalOutput")

    with tile.TileContext(nc) as tc:
        with (
            tc.tile_pool(name="const", bufs=2) as const_pool,
            tc.tile_pool(name="w1", bufs=W1C) as w1_pool,
            tc.tile_pool(name="w2", bufs=W2C) as w2_pool,
            tc.tile_pool(name="xg", bufs=2) as xg_pool,
            tc.tile_pool(name="hid", bufs=1) as hid_pool,
            tc.tile_pool(name="yt", bufs=3) as y_pool,
            tc.tile_pool(name="ps1", bufs=3, space="PSUM") as ps1_pool,
            tc.tile_pool(name="ps2", bufs=3, space="PSUM") as ps2_pool,
        ):
          for rep in range(reps):
            # first xg block first: compute starts as soon as it and the
            # first w1 chunk land; everything else hides under compute.
            xg0 = xg_pool.tile([P, H // P, min(BLK, capA)], BF16, tag="xg")
            nc.sync.dma_start(out=xg0[:], in_=xg_d[:, :, 0:min(BLK, capA)])
            cv_sb = const_pool.tile([P, cap // P], F32, tag="cv")
            nc.sync.dma_start(out=cv_sb[:], in_=cv_d[:])

            m_per_chunk = (FF // W1C) // P          # 8 m-tiles per w1 chunk
            for s, (sbase, scap, snreal) in enumerate(segs):
                # this segment's weights (SBUF-resident while it runs; the
                # pool slots make the next segment's loads wait for the
                # current segment's last reads, so they stream in behind)
                w1c = []
                for mc in range(W1C):
                    t = w1_pool.tile([P, H // P, FF // W1C], BF16, tag="w1")
                    nc.sync.dma_start(out=t[:], in_=w1_d[s, mc])
                    w1c.append(t)
                w2c = []
                for hc in range(W2C):
                    t = w2_pool.tile([P, FF // P, H // W2C], BF16, tag="w2")
                    nc.sync.dma_start(out=t[:], in_=w2_d[s, hc])
                    w2c.append(t)
                b1_sb = const_pool.tile([P, FF // P], F32, tag="b1")
                nc.sync.dma_start(out=b1_sb[:], in_=b1_d[s])

                nblk = (scap + BLK - 1) // BLK
                for b in range(nblk):
                    b0 = b * BLK
                    w = min(BLK, scap - b0)
                    we = min(w, max(snreal - b0, 0))     # real columns
                    g0 = sbase + b0                      # global slot base
                    if s == 0 and b == 0:
                        xg_t = xg0
                    else:
                        xg_t = xg_pool.tile([P, H // P, w], BF16, tag="xg")
                        nc.sync.dma_start(out=xg_t[:],
                                          in_=xg_d[:, :, g0:g0 + w])

                    # ---- mm1: hid^T = gelu(w1.T @ xg_blk + b1) ----
                    hid = hid_pool.tile([P, FF // P, w], BF16, tag="hid")
                    jt_n = (we + P - 1) // P         # live mm2 token-tiles
                    if we < jt_n * P:
                        nc.vector.memset(hid[:, :, we:jt_n * P], 0.0)
                    for m in range(FF // P):
                        if we == 0:
                            break
                        mc, mi = divmod(m, m_per_chunk)
                        ps = ps1_pool.tile([P, we], F32, tag="ps1")
                        for k in range(H // P):
                            nc.tensor.matmul(
                                ps[:],
                                lhsT=w1c[mc][:, k, mi * P:(mi + 1) * P],
                                rhs=xg_t[:, k, :we],
                                start=(k == 0),
                                stop=(k == H // P - 1),
                            )
                        nc.scalar.activation(
                            hid[:, m, :we], ps[:], act_fn,
                            bias=b1_sb[:, m:m + 1])

                    # ---- mm2 (token-major): y[tok, h] = hid^T.T @ w2 ----
                    for jt in range((we + P - 1) // P):
                        for hc in range(W2C):
                            ps = ps2_pool.tile([P, H // W2C], F32, tag="ps2")
                            for k in range(FF // P):
                                nc.tensor.matmul(
                                    ps[:],
                                    lhsT=hid[:, k, jt * P:(jt + 1) * P],
                                    rhs=w2c[hc][:, k, :],
                                    start=(k == 0),
                                    stop=(k == FF // P - 1),
                                )
                            yt = y_pool.tile([P, H // W2C], F32, tag="yt")
                            nc.vector.tensor_scalar_mul(
                                yt[:], ps[:],
                                cv_sb[:, g0 // P + jt:g0 // P + jt + 1])
                            nc.sync.dma_start(
                                out=y_d[g0 + jt * P:g0 + (jt + 1) * P,
                                        hc * (H // W2C):(hc + 1) * (H // W2C)],
                                in_=yt[:])

    nc.compile()
    return nc


# ----------------------------------------------------------------------------
# Entry point
# ----------------------------------------------------------------------------

def prepare(x, router_w, router_b, w1, b1, w2, b2):
    """Host-side sharding: returns (nc, in_maps, combine_info)."""
    x_flat = np.ascontiguousarray(np.asarray(x, np.float32).reshape(T, H))
    e1, e2, c1, c2 = _route(x_flat, np.asarray(router_w), np.asarray(router_b))
    w1 = np.asarray(w1, np.float32)
    b1 = np.asarray(b1, np.float32)
    w2 = np.asarray(w2, np.float32)
    b2 = np.asarray(b2, np.float32)

    # Group routed entries by expert (token order within an expert).
    ee = np.concatenate([e1, e2])                      # [2T] expert of entry
    cc = np.concatenate([c1, c2])                      # [2T] combine weight
    tt = np.concatenate([np.arange(T), np.arange(T)])  # [2T] token of entry
    order = np.argsort(ee, kind="stable")              # entries by expert
    counts = np.bincount(ee, minlength=E)
    starts = np.zeros(E + 1, np.int64)
    starts[1:] = np.cumsum(counts)

    capA, capB, slotsA, slotsB = _pack(counts)
    cap = capA + capB
    nrealA = max(sz for (_, _, sz) in slotsA)
    nrealB = max(sz for (_, _, sz) in slotsB) if capB else 0

    # Global Y row of each entry: core*cap + seg base + offset in piece.
    slot_of_entry = np.empty(2 * T, np.int64)
    x_flat_bf = x_flat.astype(NP_BF16)
    in_maps = []
    for c in range(N_CORES):
        xgT = np.zeros((H, cap), NP_BF16)
        cvals = np.zeros(cap, np.float32)
        w1b = np.zeros((2 if capB else 1, W1C, P, H // P, FF // W1C), NP_BF16)
        w2b = np.zeros((2 if capB else 1, W2C, P, FF // P, H // W2C), NP_BF16)
        b1f = np.zeros((2 if capB else 1, P, FF // P), np.float32)
        pieces = [(0, slotsA[c])] + ([(capA, slotsB[c])] if capB else [])
        for sbase, (e, st, sz) in pieces:
            if sz > 0:
                ent = order[starts[e] + st:starts[e] + st + sz]
                xgT[:, sbase:sbase + sz] = x_flat_bf[tt[ent]].T
                cvals[sbase:sbase + sz] = cc[ent]
                slot_of_entry[ent] = c * cap + sbase + np.arange(sz)
            s = 0 if sbase == 0 else 1
            w1b[s] = (w1[e].astype(NP_BF16)
                      .reshape(H // P, P, W1C, FF // W1C).transpose(2, 1, 0, 3))
            w2b[s] = (w2[e].astype(NP_BF16)
                      .reshape(FF // P, P, W2C, H // W2C).transpose(2, 1, 0, 3))
            b1f[s] = b1[e].reshape(FF // P, P).T
        xg = np.ascontiguousarray(
            xgT.reshape(H // P, P, cap).transpose(1, 0, 2))
        cv = np.ascontiguousarray(cvals.reshape(cap // P, P).T)
        in_maps.append(dict(xg=xg, w1b=np.ascontiguousarray(w1b),
                            w2b=np.ascontiguousarray(w2b), b1f=b1f, cv=cv))

    idx1, idx2 = slot_of_entry[:T], slot_of_entry[T:]
    # Exact f32 bias contribution (sum of combine weights is 1 per token).
    bias_add = c1[:, None] * b2[e1] + c2[:, None] * b2[e2]
    combine_info = (cap, idx1, idx2, bias_add)

    key = (capA, nrealA, capB, nrealB)
    global _PROGRAM_KEY
    _PROGRAM_KEY = key
    if key not in _PROGRAM_CACHE:
        _PROGRAM_CACHE[key] = build_program(key)
    return _PROGRAM_CACHE[key], in_maps, combine_info


def kernel(x, router_w, router_b, w1, b1, w2, b2):
    nc, in_maps, (cap, idx1, idx2, bias_add) = prepare(
        x, router_w, router_b, w1, b1, w2, b2)
    res = run_bass_kernel_spmd(nc, in_maps, core_ids=list(range(N_CORES)))
    Y = np.concatenate([res.results[c]["y"] for c in range(N_CORES)], axis=0)
    out_full = Y[idx1] + Y[idx2] + bias_add
    return out_full.reshape(B, SEQ, H).astype(np.float32)
